# revision 13
# baseline (speedup 1.0000x reference)
"""AuxSpatialGather (per-class masked mean pooling) Trainium2 kernel.

Computes, per sample b:  ctx[k, c] = mean over pixels n with gt[n]==k of feats[c, n]
(classes with zero pixels get 0), returned as [B, C, K, 1] float32.

Strategy (8 NeuronCores, data-parallel over batch, 2 samples/core):
  - feats arrive channel-major [C, HW]; the PE matmul contracts over the
    partition dim, so feats must become pixel-major on chip. fp32 matmul on
    TRN2 runs at 1/4 rate, so we cast fp32->fp16 on DVE after a plain f32
    HWDGE load (SWDGE cast-DMA measured ~3x slower per SDMA engine), then
    PE-transpose PAIRS of fp16 pixels viewed as one f32 element (halves the
    transpose count), evacuate PSUM->SBUF, and run the one-hot matmul in fp16
    (two parity-split matmuls over a stride-2 rhs view) with fp32 PSUM
    accumulation. Only precision loss: fp16 input quantization.
  - one-hot weights are built on-chip from gt via is_equal against constants,
    in the pair-interleaved pixel order matching the transposes.
  - per-class counts via a free-dim reduce + ones-vector matmul; the final
    [19, 512] context is scaled by 1/max(cnt,1) and transposed to [512, 19].
"""

import numpy as np

NUM_CLASSES = 19
B, C, H, W = 16, 512, 128, 128
HW = H * W
N_CORES = 8
S = B // N_CORES  # samples per core
P = 128  # partitions

_compiled = None


def _build_nc(s=S, c=C, hw=HW, qw=4096):
    from concourse import bacc, mybir
    from concourse.tile import TileContext
    from concourse.masks import make_identity

    f32 = mybir.dt.float32
    f16 = mybir.dt.float16
    i32 = mybir.dt.int32
    K = NUM_CLASSES
    n_ci = c // P  # channel tiles (4)
    n_q = hw // qw  # n-chunks per sample (4)
    n_j = qw // 256  # pair-windows (256 pixels) per chunk (16)
    n_t = hw // P  # 128-pixel weight columns per sample (128)
    n_u = 4  # load quarters for the startup chunk

    nc = bacc.Bacc("TRN2", target_bir_lowering=False)
    feats = nc.dram_tensor("feats", [s, c, hw], f32, kind="ExternalInput")
    gt = nc.dram_tensor("gt_seg_map", [s, hw], i32, kind="ExternalInput")
    out = nc.dram_tensor("out", [s, c, K], f32, kind="ExternalOutput")

    with TileContext(nc) as tc:
        with (
            tc.tile_pool(name="const", bufs=1) as const_pool,
            tc.tile_pool(name="stage", bufs=4) as stage_pool,
            tc.tile_pool(name="chunks", bufs=3) as chunk_pool,
            tc.tile_pool(name="planes", bufs=2) as plane_pool,
            tc.tile_pool(name="ft", bufs=4) as ft_pool,
            tc.tile_pool(name="small", bufs=2) as small_pool,
            tc.tile_pool(name="ftp", bufs=3, space="PSUM") as ftp_pool,
            tc.tile_pool(name="accp", bufs=2, space="PSUM") as acc_pool,
            tc.tile_pool(name="tinyp", bufs=1, space="PSUM") as tiny_pool,
        ):
            ident32 = const_pool.tile([P, P], f32)
            make_identity(nc, ident32[:])
            ones16 = const_pool.tile([P, 1], f16)
            nc.vector.memset(ones16[:], 1.0)

            # Pixel orders per chunk q (must match transposes, G layout, W):
            #  q == 0 (startup, contiguous pair-windows so the first window
            #          only needs the first quarter of each channel tile):
            #     n = 256*j + 2*p + par,        t = 2*j + par
            #  q >= 1 (stride-16 pair-windows so gt loads in 128B runs):
            #     n = q*qw + 32*p + 2*j + par,  t = q*32 + 2*j + par

            def load_chunks(si, q, split):
                """f32 loads + DVE casts for (si, q); split quarters the
                loads/casts (startup), else one 2MB DMA per channel tile."""
                uw = qw // n_u
                chs = []
                for ci in range(n_ci):
                    chs.append(chunk_pool.tile([P, qw], f16, name=f"ch{ci}"))
                pieces = (
                    [(ci, u) for u in range(n_u) for ci in range(n_ci)]
                    if split else [(ci, None) for ci in range(n_ci)]
                )
                for ci, u in pieces:
                    sl = slice(u * uw, (u + 1) * uw) if u is not None else slice(0, qw)
                    st = stage_pool.tile([P, sl.stop - sl.start], f32, name="st")
                    nc.sync.dma_start(
                        out=st[:],
                        in_=feats[
                            si, ci * P : (ci + 1) * P,
                            q * qw + sl.start : q * qw + sl.stop,
                        ],
                    )
                    nc.vector.tensor_copy(chs[ci][:, sl], st[:])
                return chs

            def build_planes(si):
                """One-hot planes for sample si, in the per-q pixel orders."""
                G_i = plane_pool.tile([P, n_t], i32, name="G_i")
                # both on the second HWDGE ring (ACT) - off the feat-load FIFO.
                # q0 columns: t = 2j+par -> runs of 2 (slower descriptors, 16KB)
                nc.scalar.dma_start(
                    out=G_i[:, : 2 * n_j].rearrange("p (j two) -> p j two", two=2),
                    in_=gt[si, :qw].rearrange("(j p two) -> p j two", p=P, two=2),
                )
                # q>=1 columns: runs of 32 -> fast
                nc.scalar.dma_start(
                    out=G_i[:, 2 * n_j :].rearrange("p (q r) -> p q r", q=n_q - 1),
                    in_=gt[si, qw:].rearrange("(q p r) -> p q r", q=n_q - 1, p=P),
                )
                G_f = plane_pool.tile([P, n_t], f16, name="G_f")
                nc.vector.tensor_copy(G_f[:], G_i[:])
                planes = plane_pool.tile([P, K * n_t], f16, name="planes")
                for k in range(K):
                    nc.vector.tensor_scalar(
                        planes[:, k * n_t : (k + 1) * n_t],
                        G_f[:],
                        float(k),
                        None,
                        op0=mybir.AluOpType.is_equal,
                    )
                return planes

            def build_recip(planes):
                """Per-class counts -> reciprocal [K, 1]."""
                partial = small_pool.tile([P, K], f32, name="partial")
                nc.vector.tensor_reduce(
                    partial[:],
                    planes[:].rearrange("p (k t) -> p k t", k=K),
                    axis=mybir.AxisListType.X,
                    op=mybir.AluOpType.add,
                )
                partial16 = small_pool.tile([P, K], f16, name="partial16")
                nc.vector.tensor_copy(partial16[:], partial[:])
                cnt_ps = tiny_pool.tile([1, K], f32, name="cnt_ps")
                nc.tensor.matmul(
                    cnt_ps[:], ones16[:], partial16[:], start=True, stop=True
                )
                cnt_sq = small_pool.tile([32, 32], f32, name="cnt_sq")
                nc.vector.memset(cnt_sq[:], 0.0)
                nc.vector.tensor_copy(cnt_sq[:1, :K], cnt_ps[:])
                cnt_tr = small_pool.tile([32, 32], f32, name="cnt_tr")
                nc.vector.transpose(cnt_tr[:], cnt_sq[:])
                recip = small_pool.tile([K, 1], f32, name="recip")
                nc.vector.tensor_scalar_max(recip[:], cnt_tr[:K, :1], 1.0)
                nc.vector.reciprocal(recip[:], recip[:])
                return recip

            # gt+planes first (tiny DMA on the ACT ring), then the first
            # quartered chunk so the first window is ready after ~0.5MB/ci
            planes_cur = build_planes(0)
            pending = load_chunks(0, 0, split=True)

            # ---- main loop: load -> cast -> pair-transpose -> matmul ----
            for si in range(s):
                acc = acc_pool.tile([K, c], f32, name="acc")
                W_all = planes_cur[:].rearrange("p (k t) -> p t k", t=n_t)
                for q in range(n_q):
                    chs = pending
                    if q + 1 < n_q:
                        pending = load_chunks(si, q + 1, split=False)
                    elif si + 1 < s:
                        pending = load_chunks(si + 1, 0, split=True)
                        planes_next = build_planes(si + 1)
                    if q == 0:
                        recip = build_recip(planes_cur)
                    for j in range(n_j):
                        ftp = ftp_pool.tile([P, c], f32, name="ftp")
                        for ci in range(n_ci):
                            win = (
                                chs[ci][:].bitcast(f32)[:, j * P : (j + 1) * P]
                                if q == 0
                                else chs[ci][:].bitcast(f32)[
                                    :, j : j + (P - 1) * n_j + 1 : n_j
                                ]
                            )
                            nc.tensor.transpose(
                                ftp[:, ci * P : (ci + 1) * P], win, ident32[:]
                            )
                        fts = ft_pool.tile([P, 2 * c], f16, name="fts")
                        if j % 2 == 0:
                            nc.vector.tensor_copy(fts[:].bitcast(f32), ftp[:])
                        else:
                            nc.scalar.copy(fts[:].bitcast(f32), ftp[:])
                        fts_pairs = fts[:].rearrange("p (c two) -> p two c", two=2)
                        for par in range(2):
                            t = q * (n_j * 2) + 2 * j + par
                            nc.tensor.matmul(
                                acc[:],
                                W_all[:, t, :],
                                fts_pairs[:, par, :],
                                start=(t == 0),
                                stop=(t == n_t - 1),
                            )

                # ---- normalize + emit [c, K] ----
                final = small_pool.tile([K, c], f32, name="final")
                nc.vector.tensor_scalar(
                    final[:], acc[:], recip[:, :1], None,
                    op0=mybir.AluOpType.mult,
                )
                outT_ps = tiny_pool.tile([P, n_ci * K], f32, name="outT_ps")
                for ci in range(n_ci):
                    nc.tensor.transpose(
                        outT_ps[:, ci * K : (ci + 1) * K],
                        final[:K, ci * P : (ci + 1) * P],
                        ident32[:K, :K],
                    )
                outT = small_pool.tile([P, n_ci * K], f32, name="outT")
                nc.vector.tensor_copy(outT[:], outT_ps[:])
                # SWDGE: keep the HWDGE feat-load queue free of DMAs that
                # wait on compute (FIFO per issuing engine)
                nc.gpsimd.dma_start(
                    out=out[si].rearrange("(ci p) k -> p ci k", p=P),
                    in_=outT[:].rearrange("p (ci k) -> p ci k", k=K),
                )
                if si + 1 < s:
                    planes_cur = planes_next
    nc.compile()
    return nc


def _get_compiled():
    global _compiled
    if _compiled is None:
        _compiled = _build_nc()
    return _compiled


def kernel(feats, gt_seg_map):
    from concourse.bass_utils import run_bass_kernel_spmd

    feats = np.asarray(feats, dtype=np.float32).reshape(B, C, HW)
    gt = np.asarray(gt_seg_map).astype(np.int32).reshape(B, HW)

    nc = _get_compiled()
    in_maps = []
    for i in range(N_CORES):
        in_maps.append(
            {
                "feats": feats[i * S : (i + 1) * S],
                "gt_seg_map": gt[i * S : (i + 1) * S],
            }
        )
    res = run_bass_kernel_spmd(nc, in_maps, core_ids=list(range(N_CORES)))
    parts = [res.results[i]["out"] for i in range(N_CORES)]  # each [S, C, K]
    full = np.concatenate(parts, axis=0)  # [B, C, K]
    return full[..., None].astype(np.float32)  # [B, C, K, 1]


# revision 14
# speedup vs baseline: 1.0944x; 1.0944x over previous
"""AuxSpatialGather (per-class masked mean pooling) Trainium2 kernel.

Computes, per sample b:  ctx[k, c] = mean over pixels n with gt[n]==k of feats[c, n]
(classes with zero pixels get 0), returned as [B, C, K, 1] float32.

Strategy (8 NeuronCores, data-parallel over batch, 2 samples/core):
  - feats arrive channel-major [C, HW]; the PE matmul contracts over the
    partition dim, so feats must become pixel-major on chip. fp32 matmul on
    TRN2 runs at 1/4 rate, so we cast fp32->fp16 on DVE after a plain f32
    HWDGE load (SWDGE cast-DMA measured ~3x slower per SDMA engine), then
    PE-transpose PAIRS of fp16 pixels viewed as one f32 element (halves the
    transpose count), evacuate PSUM->SBUF, and run the one-hot matmul in fp16
    (two parity-split matmuls over a stride-2 rhs view) with fp32 PSUM
    accumulation. Only precision loss: fp16 input quantization.
  - one-hot weights are built on-chip from gt via is_equal against constants,
    in the pair-interleaved pixel order matching the transposes.
  - per-class counts via a free-dim reduce + ones-vector matmul; the final
    [19, 512] context is scaled by 1/max(cnt,1) and transposed to [512, 19].
"""

import numpy as np

NUM_CLASSES = 19
B, C, H, W = 16, 512, 128, 128
HW = H * W
N_CORES = 8
S = B // N_CORES  # samples per core
P = 128  # partitions

_compiled = None


def _build_nc(s=S, c=C, hw=HW, qw=4096):
    from concourse import bacc, mybir
    from concourse.tile import TileContext
    from concourse.masks import make_identity

    f32 = mybir.dt.float32
    f16 = mybir.dt.float16
    i32 = mybir.dt.int32
    K = NUM_CLASSES
    n_ci = c // P  # channel tiles (4)
    n_q = hw // qw  # n-chunks per sample (4)
    n_j = qw // 256  # pair-windows (256 pixels) per chunk (16)
    n_t = hw // P  # 128-pixel weight columns per sample (128)
    n_u = 4  # load quarters for the startup chunk

    nc = bacc.Bacc("TRN2", target_bir_lowering=False)
    feats = nc.dram_tensor("feats", [s, c, hw], f32, kind="ExternalInput")
    gt = nc.dram_tensor("gt_seg_map", [s, hw], i32, kind="ExternalInput")
    out = nc.dram_tensor("out", [s, c, K], f32, kind="ExternalOutput")

    with TileContext(nc) as tc:
        with (
            tc.tile_pool(name="const", bufs=1) as const_pool,
            tc.tile_pool(name="stage", bufs=4) as stage_pool,
            tc.tile_pool(name="chunks", bufs=3) as chunk_pool,
            tc.tile_pool(name="planes", bufs=2) as plane_pool,
            tc.tile_pool(name="ft", bufs=4) as ft_pool,
            tc.tile_pool(name="small", bufs=2) as small_pool,
            tc.tile_pool(name="ftp", bufs=3, space="PSUM") as ftp_pool,
            tc.tile_pool(name="accp", bufs=2, space="PSUM") as acc_pool,
            tc.tile_pool(name="tinyp", bufs=1, space="PSUM") as tiny_pool,
        ):
            ident32 = const_pool.tile([P, P], f32)
            make_identity(nc, ident32[:])
            ones16 = const_pool.tile([P, 1], f16)
            nc.vector.memset(ones16[:], 1.0)

            # Pixel order (all chunks): n = q*qw + 32*p + 2*j + par
            # -> G[p, t], t = q*32 + 2j + par: per-partition runs of 32
            # contiguous gt elements -> fast gt DMA; transpose windows are
            # stride-n_j pair columns.

            def load_chunks(si, q, split):
                """f32 loads + DVE casts for (si, q); split halves the loads
                (startup), else one 2MB DMA per channel tile."""
                chs = []
                for ci in range(n_ci):
                    st = stage_pool.tile([P, qw], f32, name="st")
                    ch = chunk_pool.tile([P, qw], f16, name=f"ch{ci}")
                    halves = 2 if split else 1
                    hw_half = qw // halves
                    for h in range(halves):
                        sl = slice(h * hw_half, (h + 1) * hw_half)
                        nc.sync.dma_start(
                            out=st[:, sl],
                            in_=feats[
                                si,
                                ci * P : (ci + 1) * P,
                                q * qw + h * hw_half : q * qw + (h + 1) * hw_half,
                            ],
                        )
                        nc.vector.tensor_copy(ch[:, sl], st[:, sl])
                    chs.append(ch)
                return chs

            def build_planes(si):
                """One-hot planes for sample si (pair-order pixel layout)."""
                G_i = plane_pool.tile([P, n_t], i32, name="G_i")
                # second HWDGE ring (ACT): off the FIFO ring feeding feat loads
                nc.scalar.dma_start(
                    out=G_i[:].rearrange("p (q r) -> p q r", q=n_q),
                    in_=gt[si].rearrange("(q p r) -> p q r", q=n_q, p=P),
                )
                G_f = plane_pool.tile([P, n_t], f16, name="G_f")
                nc.vector.tensor_copy(G_f[:], G_i[:])
                planes = plane_pool.tile([P, K * n_t], f16, name="planes")
                for k in range(K):
                    nc.vector.tensor_scalar(
                        planes[:, k * n_t : (k + 1) * n_t],
                        G_f[:],
                        float(k),
                        None,
                        op0=mybir.AluOpType.is_equal,
                    )
                return planes

            def build_recip(planes):
                """Per-class counts -> reciprocal [K, 1]."""
                partial = small_pool.tile([P, K], f32, name="partial")
                nc.vector.tensor_reduce(
                    partial[:],
                    planes[:].rearrange("p (k t) -> p k t", k=K),
                    axis=mybir.AxisListType.X,
                    op=mybir.AluOpType.add,
                )
                partial16 = small_pool.tile([P, K], f16, name="partial16")
                nc.vector.tensor_copy(partial16[:], partial[:])
                cnt_ps = tiny_pool.tile([1, K], f32, name="cnt_ps")
                nc.tensor.matmul(
                    cnt_ps[:], ones16[:], partial16[:], start=True, stop=True
                )
                cnt_sq = small_pool.tile([32, 32], f32, name="cnt_sq")
                nc.vector.memset(cnt_sq[:], 0.0)
                nc.vector.tensor_copy(cnt_sq[:1, :K], cnt_ps[:])
                cnt_tr = small_pool.tile([32, 32], f32, name="cnt_tr")
                nc.vector.transpose(cnt_tr[:], cnt_sq[:])
                recip = small_pool.tile([K, 1], f32, name="recip")
                nc.vector.tensor_scalar_max(recip[:], cnt_tr[:K, :1], 1.0)
                nc.vector.reciprocal(recip[:], recip[:])
                return recip

            # gt+planes first (tiny DMA on the ACT ring), then the first
            # quartered chunk so the first window is ready after ~0.5MB/ci
            planes_cur = build_planes(0)
            pending = load_chunks(0, 0, split=True)

            # ---- main loop: load -> cast -> pair-transpose -> matmul ----
            for si in range(s):
                acc = acc_pool.tile([K, c], f32, name="acc")
                W_all = planes_cur[:].rearrange("p (k t) -> p t k", t=n_t)
                for q in range(n_q):
                    chs = pending
                    if q + 1 < n_q:
                        pending = load_chunks(si, q + 1, split=False)
                    elif si + 1 < s:
                        pending = load_chunks(si + 1, 0, split=False)
                        planes_next = build_planes(si + 1)
                    for j in range(n_j):
                        ftp = ftp_pool.tile([P, c], f32, name="ftp")
                        for ci in range(n_ci):
                            nc.tensor.transpose(
                                ftp[:, ci * P : (ci + 1) * P],
                                chs[ci][:].bitcast(f32)[
                                    :, j : j + (P - 1) * n_j + 1 : n_j
                                ],
                                ident32[:],
                            )
                        fts = ft_pool.tile([P, 2 * c], f16, name="fts")
                        if j % 2 == 0:
                            nc.vector.tensor_copy(fts[:].bitcast(f32), ftp[:])
                        else:
                            nc.scalar.copy(fts[:].bitcast(f32), ftp[:])
                        fts_pairs = fts[:].rearrange("p (c two) -> p two c", two=2)
                        for par in range(2):
                            t = q * (n_j * 2) + 2 * j + par
                            nc.tensor.matmul(
                                acc[:],
                                W_all[:, t, :],
                                fts_pairs[:, par, :],
                                start=(t == 0),
                                stop=(t == n_t - 1),
                            )

                # ---- normalize + emit [c, K] ----
                recip = build_recip(planes_cur)
                final = small_pool.tile([K, c], f32, name="final")
                nc.vector.tensor_scalar(
                    final[:], acc[:], recip[:, :1], None,
                    op0=mybir.AluOpType.mult,
                )
                outT_ps = tiny_pool.tile([P, n_ci * K], f32, name="outT_ps")
                for ci in range(n_ci):
                    nc.tensor.transpose(
                        outT_ps[:, ci * K : (ci + 1) * K],
                        final[:K, ci * P : (ci + 1) * P],
                        ident32[:K, :K],
                    )
                outT = small_pool.tile([P, n_ci * K], f32, name="outT")
                nc.vector.tensor_copy(outT[:], outT_ps[:])
                # SWDGE: keep the HWDGE feat-load queue free of DMAs that
                # wait on compute (FIFO per issuing engine)
                nc.gpsimd.dma_start(
                    out=out[si].rearrange("(ci p) k -> p ci k", p=P),
                    in_=outT[:].rearrange("p (ci k) -> p ci k", k=K),
                )
                if si + 1 < s:
                    planes_cur = planes_next
    nc.compile()
    return nc


def _get_compiled():
    global _compiled
    if _compiled is None:
        _compiled = _build_nc()
    return _compiled


def kernel(feats, gt_seg_map):
    from concourse.bass_utils import run_bass_kernel_spmd

    feats = np.asarray(feats, dtype=np.float32).reshape(B, C, HW)
    gt = np.asarray(gt_seg_map).astype(np.int32).reshape(B, HW)

    nc = _get_compiled()
    in_maps = []
    for i in range(N_CORES):
        in_maps.append(
            {
                "feats": feats[i * S : (i + 1) * S],
                "gt_seg_map": gt[i * S : (i + 1) * S],
            }
        )
    res = run_bass_kernel_spmd(nc, in_maps, core_ids=list(range(N_CORES)))
    parts = [res.results[i]["out"] for i in range(N_CORES)]  # each [S, C, K]
    full = np.concatenate(parts, axis=0)  # [B, C, K]
    return full[..., None].astype(np.float32)  # [B, C, K, 1]


# revision 15
# speedup vs baseline: 1.2747x; 1.1648x over previous
"""AuxSpatialGather (per-class masked mean pooling) Trainium2 kernel.

Computes, per sample b:  ctx[k, c] = mean over pixels n with gt[n]==k of feats[c, n]
(classes with zero pixels get 0), returned as [B, C, K, 1] float32.

Strategy (8 NeuronCores, data-parallel over batch, 2 samples/core):
  - feats arrive channel-major [C, HW]; the PE matmul contracts over the
    partition dim, so feats must become pixel-major on chip. fp32 matmul on
    TRN2 runs at ~1/4 rate, so feats are cast fp32->fp16 on DVE after plain
    f32 HWDGE loads (SWDGE cast-DMA measured ~3x slower per SDMA engine),
    then PE-transposed as PAIRS of fp16 pixels viewed as one f32 element
    (halves the transpose count; PE transpose-mode is a bit-exact raw mover),
    evacuated PSUM->SBUF (DVE/ACT alternating), and reduced by a one-hot
    matmul in fp16 (two parity-split matmuls over a stride-2 rhs view) with
    fp32 PSUM accumulation. Only precision loss: fp16 input quantization.
  - pair-windows use stride-n_j columns so the gt load lands in 32-element
    contiguous runs (fast DMA on the second HWDGE ring, off the feat FIFO).
  - transposes are emitted ci-major in groups of 4 windows so PE only needs
    the first channel tile of a chunk to start working on it: its idle at
    chunk boundaries stays under the ~3.4us HAM re-throttle window.
  - per-class counts via a free-dim reduce + ones-vector matmul; the final
    [19, 512] context is scaled by 1/max(cnt,1) and transposed to [512, 19].
"""

import numpy as np

NUM_CLASSES = 19
B, C, H, W = 16, 512, 128, 128
HW = H * W
N_CORES = 8
S = B // N_CORES  # samples per core
P = 128  # partitions

_compiled = None


def _build_nc(s=S, c=C, hw=HW, qw=4096):
    from concourse import bacc, mybir
    from concourse.tile import TileContext
    from concourse.masks import make_identity

    f32 = mybir.dt.float32
    f16 = mybir.dt.float16
    i32 = mybir.dt.int32
    K = NUM_CLASSES
    n_ci = c // P  # channel tiles (4)
    n_q = hw // qw  # n-chunks per sample (4)
    n_j = qw // 256  # pair-windows (256 pixels) per chunk (16)
    n_t = hw // P  # 128-pixel weight columns per sample (128)
    n_u = 4  # load quarters for the startup chunk

    nc = bacc.Bacc("TRN2", target_bir_lowering=False)
    feats = nc.dram_tensor("feats", [s, c, hw], f32, kind="ExternalInput")
    gt = nc.dram_tensor("gt_seg_map", [s, hw], i32, kind="ExternalInput")
    out = nc.dram_tensor("out", [s, c, K], f32, kind="ExternalOutput")

    with TileContext(nc) as tc:
        with (
            tc.tile_pool(name="const", bufs=1) as const_pool,
            tc.tile_pool(name="stage", bufs=4) as stage_pool,
            tc.tile_pool(name="chunks", bufs=3) as chunk_pool,
            tc.tile_pool(name="planes", bufs=2) as plane_pool,
            tc.tile_pool(name="ft", bufs=4) as ft_pool,
            tc.tile_pool(name="small", bufs=2) as small_pool,
            tc.tile_pool(name="ftp", bufs=5, space="PSUM") as ftp_pool,
            tc.tile_pool(name="accp", bufs=2, space="PSUM") as acc_pool,
            tc.tile_pool(name="tinyp", bufs=1, space="PSUM") as tiny_pool,
        ):
            ident32 = const_pool.tile([P, P], f32)
            make_identity(nc, ident32[:])
            ones16 = const_pool.tile([P, 1], f16)
            nc.vector.memset(ones16[:], 1.0)

            # Pixel order (all chunks): n = q*qw + 32*p + 2*j + par
            # -> G[p, t], t = q*32 + 2j + par: per-partition runs of 32
            # contiguous gt elements -> fast gt DMA; transpose windows are
            # stride-n_j pair columns.

            def load_chunks(si, q, split):
                """f32 loads + DVE casts for (si, q); split halves the loads
                (startup), else one 2MB DMA per channel tile."""
                chs = []
                for ci in range(n_ci):
                    st = stage_pool.tile([P, qw], f32, name="st")
                    ch = chunk_pool.tile([P, qw], f16, name=f"ch{ci}")
                    halves = 2 if split else 1
                    hw_half = qw // halves
                    for h in range(halves):
                        sl = slice(h * hw_half, (h + 1) * hw_half)
                        nc.sync.dma_start(
                            out=st[:, sl],
                            in_=feats[
                                si,
                                ci * P : (ci + 1) * P,
                                q * qw + h * hw_half : q * qw + (h + 1) * hw_half,
                            ],
                        )
                        nc.vector.tensor_copy(ch[:, sl], st[:, sl])
                    chs.append(ch)
                return chs

            def build_planes(si):
                """One-hot planes for sample si (pair-order pixel layout)."""
                G_i = plane_pool.tile([P, n_t], i32, name="G_i")
                # second HWDGE ring (ACT): off the FIFO ring feeding feat loads
                nc.scalar.dma_start(
                    out=G_i[:].rearrange("p (q r) -> p q r", q=n_q),
                    in_=gt[si].rearrange("(q p r) -> p q r", q=n_q, p=P),
                )
                G_f = plane_pool.tile([P, n_t], f16, name="G_f")
                nc.vector.tensor_copy(G_f[:], G_i[:])
                planes = plane_pool.tile([P, K * n_t], f16, name="planes")
                for k in range(K):
                    nc.vector.tensor_scalar(
                        planes[:, k * n_t : (k + 1) * n_t],
                        G_f[:],
                        float(k),
                        None,
                        op0=mybir.AluOpType.is_equal,
                    )
                return planes

            def build_recip(planes):
                """Per-class counts -> reciprocal [K, 1]."""
                partial = small_pool.tile([P, K], f32, name="partial")
                nc.vector.tensor_reduce(
                    partial[:],
                    planes[:].rearrange("p (k t) -> p k t", k=K),
                    axis=mybir.AxisListType.X,
                    op=mybir.AluOpType.add,
                )
                partial16 = small_pool.tile([P, K], f16, name="partial16")
                nc.vector.tensor_copy(partial16[:], partial[:])
                cnt_ps = tiny_pool.tile([1, K], f32, name="cnt_ps", tag="tiny")
                nc.tensor.matmul(
                    cnt_ps[:], ones16[:], partial16[:], start=True, stop=True
                )
                cnt_sq = small_pool.tile([32, 32], f32, name="cnt_sq")
                nc.vector.memset(cnt_sq[:], 0.0)
                nc.vector.tensor_copy(cnt_sq[:1, :K], cnt_ps[:])
                cnt_tr = small_pool.tile([32, 32], f32, name="cnt_tr")
                nc.vector.transpose(cnt_tr[:], cnt_sq[:])
                recip = small_pool.tile([K, 1], f32, name="recip")
                nc.vector.tensor_scalar_max(recip[:], cnt_tr[:K, :1], 1.0)
                nc.vector.reciprocal(recip[:], recip[:])
                return recip

            # gt+planes first (tiny DMA on the ACT ring), then the first
            # quartered chunk so the first window is ready after ~0.5MB/ci
            planes_cur = build_planes(0)
            pending = load_chunks(0, 0, split=True)

            # ---- main loop: load -> cast -> pair-transpose -> matmul ----
            for si in range(s):
                acc = acc_pool.tile([K, c], f32, name="acc")
                W_all = planes_cur[:].rearrange("p (k t) -> p t k", t=n_t)
                for q in range(n_q):
                    chs = pending
                    if q + 1 < n_q:
                        pending = load_chunks(si, q + 1, split=False)
                    elif si + 1 < s:
                        pending = load_chunks(si + 1, 0, split=False)
                        planes_next = build_planes(si + 1)
                    if q == 0:
                        recip = build_recip(planes_cur)
                    for g in range(n_j // 4):
                        # ci-major transposes within a group of 4 windows:
                        # PE needs only chunk ci0 to start this group, so its
                        # idle at chunk boundaries is spread into slivers that
                        # never trip the HAM re-throttle window.
                        ftps = [
                            ftp_pool.tile([P, c], f32, name=f"ftp{jj}", tag="ftp")
                            for jj in range(4)
                        ]
                        for ci in range(n_ci):
                            for jj in range(4):
                                j = g * 4 + jj
                                nc.tensor.transpose(
                                    ftps[jj][:, ci * P : (ci + 1) * P],
                                    chs[ci][:].bitcast(f32)[
                                        :, j : j + (P - 1) * n_j + 1 : n_j
                                    ],
                                    ident32[:],
                                )
                        for jj in range(4):
                            j = g * 4 + jj
                            fts = ft_pool.tile([P, 2 * c], f16, name="fts")
                            if j % 2 == 0:
                                nc.vector.tensor_copy(fts[:].bitcast(f32), ftps[jj][:])
                            else:
                                nc.scalar.copy(fts[:].bitcast(f32), ftps[jj][:])
                            fts_pairs = fts[:].rearrange("p (c two) -> p two c", two=2)
                            for par in range(2):
                                t = q * (n_j * 2) + 2 * j + par
                                nc.tensor.matmul(
                                    acc[:],
                                    W_all[:, t, :],
                                    fts_pairs[:, par, :],
                                    start=(t == 0),
                                    stop=(t == n_t - 1),
                                )

                # ---- normalize + emit [c, K] ----
                final = small_pool.tile([K, c], f32, name="final")
                nc.vector.tensor_scalar(
                    final[:], acc[:], recip[:, :1], None,
                    op0=mybir.AluOpType.mult,
                )
                outT_ps = tiny_pool.tile([P, n_ci * K], f32, name="outT_ps", tag="tiny")
                for ci in range(n_ci):
                    nc.tensor.transpose(
                        outT_ps[:, ci * K : (ci + 1) * K],
                        final[:K, ci * P : (ci + 1) * P],
                        ident32[:K, :K],
                    )
                outT = small_pool.tile([P, n_ci * K], f32, name="outT")
                nc.vector.tensor_copy(outT[:], outT_ps[:])
                # SWDGE: keep the HWDGE feat-load queue free of DMAs that
                # wait on compute (FIFO per issuing engine)
                nc.gpsimd.dma_start(
                    out=out[si].rearrange("(ci p) k -> p ci k", p=P),
                    in_=outT[:].rearrange("p (ci k) -> p ci k", k=K),
                )
                if si + 1 < s:
                    planes_cur = planes_next
    nc.compile()
    return nc


def _get_compiled():
    global _compiled
    if _compiled is None:
        _compiled = _build_nc()
    return _compiled


def kernel(feats, gt_seg_map):
    from concourse.bass_utils import run_bass_kernel_spmd

    feats = np.asarray(feats, dtype=np.float32).reshape(B, C, HW)
    gt = np.asarray(gt_seg_map).astype(np.int32).reshape(B, HW)

    nc = _get_compiled()
    in_maps = []
    for i in range(N_CORES):
        in_maps.append(
            {
                "feats": feats[i * S : (i + 1) * S],
                "gt_seg_map": gt[i * S : (i + 1) * S],
            }
        )
    res = run_bass_kernel_spmd(nc, in_maps, core_ids=list(range(N_CORES)))
    parts = [res.results[i]["out"] for i in range(N_CORES)]  # each [S, C, K]
    full = np.concatenate(parts, axis=0)  # [B, C, K]
    return full[..., None].astype(np.float32)  # [B, C, K, 1]
